# revision 4
# baseline (speedup 1.0000x reference)
"""Trainium2 Bass kernel for nn_Attention2Context (2-context masked attention).

Self-contained: builds one SPMD Bass/Tile program, shards the FULL inputs
across 8 NeuronCores host-side (batch x 2-head groups; tensor-parallel heads:
q/k/v column-parallel, out-proj row-parallel), runs via
concourse.bass_utils.run_bass_kernel_spmd, and reduces the row-parallel
partial outputs host-side.

Math notes:
- softmax max-subtraction is skipped: logits are |sim| << 1 by construction
  (inputs scaled 0.02), masked positions are multiplied by 0 AFTER exp, which
  is exactly softmax(where(mask, sim, -inf)) when not all-masked.
- the 1/sqrt(DH) scale is folded into Wq/bq host-side.
- v biases are added on-chip (broadcast via K=1 matmul); bo is added on the
  host after the partial-sum reduction.
- all matmuls run as float32r (fp22 multiplies, fp32 accumulate).
"""

import sys

for _p in ("/opt/trn_rl_repo", "/root/.axon_site/_ro/trn_rl_repo"):
    if _p not in sys.path:
        sys.path.append(_p)

import numpy as np

P = 128
NQ = 1024
J = 2048          # NC1 + NC2
QD = 1024
NKC = 8           # contraction chunks (QD / P)
NJC = 16          # context chunks (J / P)
IT = 512          # i-tile width
NIT = NQ // IT    # 2
NHEAD_CORE = 2    # heads per core
DH = 64
N_CORES = 8

_CACHE = {}


def _build_program():
    import concourse.mybir as mybir
    import concourse.tile as tile
    from concourse import bacc

    f32 = mybir.dt.float32
    f32r = mybir.dt.float32r
    u8 = mybir.dt.uint8
    EXP = mybir.ActivationFunctionType.Exp
    MULT = mybir.AluOpType.mult
    ADD = mybir.AluOpType.add

    nc = bacc.Bacc("TRN2", target_bir_lowering=False, debug=False,
                   num_devices=N_CORES)

    def din(name, shape, dt=f32):
        return nc.dram_tensor(name, shape, dt, kind="ExternalInput").ap()

    xT = din("xT", [QD, NQ], f32r)           # x[b].T
    cT = din("cT", [QD, J], f32r)            # [ctx1[b].T | ctx2[b].T]
    maskT = din("maskT", [J, NQ], u8)        # [m1[b].T ; m2[b].T]
    wq = din("wq", [QD, P], f32r)            # pre-scaled by DH**-0.5
    wk1 = din("wk1", [QD, P], f32r)
    wk2 = din("wk2", [QD, P], f32r)
    wv1 = din("wv1", [QD, 2 * P], f32r)
    wv2 = din("wv2", [QD, 2 * P], f32r)
    wo = din("wo", [2 * P, NQ], f32r)
    bq = din("bq", [P, 1])                   # pre-scaled by DH**-0.5
    bk1 = din("bk1", [P, 1])
    bk2 = din("bk2", [P, 1])
    bv = din("bv", [1, 2 * 2 * P], f32r)     # [bv1_slice | bv2_slice]
    ones_c = din("ones_c", [P, 1], f32r)     # all-ones column
    ones_r = din("ones_r", [1, P], f32r)     # all-ones row
    out = nc.dram_tensor("out", [NQ, NQ], f32, kind="ExternalOutput").ap()


    with tile.TileContext(nc) as tc:
        with tc.tile_pool(name="persist", bufs=1) as pp:
            mask_sb = pp.tile([P, NJC, NQ], u8, tag="mask", name="mask_sb")
            qT_sb = pp.tile([P, NQ], f32r, tag="qT", name="qT_sb")
            kT_sb = pp.tile([P, J], f32r, tag="kT", name="kT_sb")
            v_sb = pp.tile([P, NJC, 2 * P], f32r, tag="v", name="v_sb")
            wo_sb = pp.tile([P, 2, NQ], f32r, tag="wo", name="wo_sb")
            outT_sb = pp.tile([P, 2, NQ], f32r, tag="outT", name="outT_sb")
            bq_sb = pp.tile([P, 1], f32, tag="bq", name="bq_sb")
            bk1_sb = pp.tile([P, 1], f32, tag="bk1", name="bk1_sb")
            bk2_sb = pp.tile([P, 1], f32, tag="bk2", name="bk2_sb")
            ones_sb = pp.tile([P, 1], f32r, tag="ones", name="ones_sb")
            onesk1_sb = pp.tile([1, P], f32r, tag="onesk1", name="onesk1_sb")
            bv_sb = pp.tile([1, 4 * P], f32r, tag="bv", name="bv_sb")
            bvb_sb = pp.tile([P, 4 * P], f32, tag="bvb", name="bvb_sb")

            for jc in range(NJC):
                nc.sync.dma_start(mask_sb[:, jc, :],
                                  maskT[jc * P:(jc + 1) * P, :])
            nc.sync.dma_start(wo_sb[:], wo.rearrange("(h p) o -> p h o", p=P))
            nc.sync.dma_start(bq_sb[:], bq)
            nc.sync.dma_start(bk1_sb[:], bk1)
            nc.sync.dma_start(bk2_sb[:], bk2)
            nc.sync.dma_start(bv_sb[:], bv)
            nc.sync.dma_start(ones_sb[:], ones_c)
            nc.sync.dma_start(onesk1_sb[:], ones_r)

            # broadcast v biases to all 128 partitions: ones_col.T @ bv_row
            with tc.tile_pool(name="psB", bufs=1, space="PSUM") as psB:
                bvb_ps = psB.tile([P, 4 * P], f32, tag="bvb_ps",
                                  name="bvb_ps")
                nc.tensor.matmul(bvb_ps[:], onesk1_sb[:], bv_sb[:],
                                 start=True, stop=True)
                nc.scalar.copy(bvb_sb[:], bvb_ps[:])

            # ---------------- phase 1: projections ----------------
            with tc.tile_pool(name="proj", bufs=1) as prj, \
                 tc.tile_pool(name="projs", bufs=3) as prjs, \
                 tc.tile_pool(name="psP", bufs=1, space="PSUM") as psP:
                cT_sb = prj.tile([P, NKC, J], f32r, tag="cT", name="cT_sb")
                wq_sb = prj.tile([P, NKC, P], f32r, tag="wq", name="wq_sb")
                wk1_sb = prj.tile([P, NKC, P], f32r, tag="wk1", name="wk1_sb")
                wk2_sb = prj.tile([P, NKC, P], f32r, tag="wk2", name="wk2_sb")
                wv1_sb = prj.tile([P, NKC, 2 * P], f32r, tag="wv1",
                                  name="wv1_sb")
                wv2_sb = prj.tile([P, NKC, 2 * P], f32r, tag="wv2",
                                  name="wv2_sb")

                nc.sync.dma_start(wq_sb[:],
                                  wq.rearrange("(kc p) m -> p kc m", p=P))
                nc.sync.dma_start(wk1_sb[:],
                                  wk1.rearrange("(kc p) m -> p kc m", p=P))
                nc.sync.dma_start(wk2_sb[:],
                                  wk2.rearrange("(kc p) m -> p kc m", p=P))
                nc.sync.dma_start(wv1_sb[:],
                                  wv1.rearrange("(kc p) m -> p kc m", p=P))
                nc.sync.dma_start(wv2_sb[:],
                                  wv2.rearrange("(kc p) m -> p kc m", p=P))
                for kc in range(NKC):
                    nc.sync.dma_start(cT_sb[:, kc, :],
                                      cT[kc * P:(kc + 1) * P, :])

                # qT / kT: kc-outer so matmuls chase the cT/xT DMAs
                q_ps = [psP.tile([P, 512], f32, tag="qk_ps", bufs=6,
                                 name=f"q_ps{nt}") for nt in range(2)]
                k_ps = [psP.tile([P, 512], f32, tag="qk_ps", bufs=6,
                                 name=f"k_ps{nt}") for nt in range(4)]
                for kc in range(NKC):
                    xt = prjs.tile([P, NQ], f32r, tag="xt", name="xt")
                    nc.sync.dma_start(xt[:], xT[kc * P:(kc + 1) * P, :])
                    for nt in range(2):
                        nc.tensor.matmul(
                            q_ps[nt][:], wq_sb[:, kc, :],
                            xt[:, nt * 512:(nt + 1) * 512],
                            start=(kc == 0), stop=(kc == NKC - 1))
                    for nt in range(4):
                        wk_sb = wk1_sb if nt < 2 else wk2_sb
                        nc.tensor.matmul(
                            k_ps[nt][:], wk_sb[:, kc, :],
                            cT_sb[:, kc, nt * 512:(nt + 1) * 512],
                            start=(kc == 0), stop=(kc == NKC - 1))
                for nt in range(2):
                    nc.vector.tensor_scalar_add(
                        qT_sb[:, nt * 512:(nt + 1) * 512], q_ps[nt][:],
                        bq_sb[:])
                for nt in range(4):
                    bk_sb = bk1_sb if nt < 2 else bk2_sb
                    nc.vector.tensor_scalar_add(
                        kT_sb[:, nt * 512:(nt + 1) * 512], k_ps[nt][:],
                        bk_sb[:])

                # v: [J, 256], j on partitions, + bias via broadcast tile
                for jc in range(NJC):
                    wv_sb = wv1_sb if jc < NJC // 2 else wv2_sb
                    bvb_half = (bvb_sb[:, 0:256] if jc < NJC // 2
                                else bvb_sb[:, 256:512])
                    v_ps = psP.tile([P, 2 * P], f32, tag="v_ps", bufs=2,
                                    name="v_ps")
                    for kc in range(NKC):
                        nc.tensor.matmul(
                            v_ps[:], cT_sb[:, kc, jc * P:(jc + 1) * P],
                            wv_sb[:, kc, :],
                            start=(kc == 0), stop=(kc == NKC - 1))
                    nc.vector.tensor_tensor(v_sb[:, jc, :], v_ps[:],
                                            bvb_half, ADD)

            # ---------------- phase 2+3: attention + out-proj ----------
            with tc.tile_pool(name="attn", bufs=1) as atp, \
                 tc.tile_pool(name="psA", bufs=1, space="PSUM") as psA:
                for it in range(NIT):
                    isl = slice(it * IT, (it + 1) * IT)
                    for h in range(NHEAD_CORE):
                        hsl = slice(h * DH, (h + 1) * DH)
                        attnT = atp.tile([P, NJC, IT], f32r, tag="attnT",
                                         bufs=2, name="attnT")
                        for jg in range(NJC // 2):
                            sim_ps = psA.tile([P, 2, IT], f32, tag="sim",
                                              bufs=2, name="sim_ps")
                            for g in range(2):
                                jc = jg * 2 + g
                                nc.tensor.matmul(
                                    sim_ps[:, g, :],
                                    kT_sb[hsl, jc * P:(jc + 1) * P],
                                    qT_sb[hsl, isl],
                                    start=True, stop=True)
                            nc.scalar.activation(
                                attnT[:, jg * 2:jg * 2 + 2, :], sim_ps[:],
                                EXP)
                        nc.vector.tensor_tensor(
                            attnT[:], attnT[:], mask_sb[:, :, isl], MULT)

                        av_ps = psA.tile([P, IT], f32, tag="av", bufs=2,
                                         name="av_ps")
                        se_ps = psA.tile([1, IT], f32, tag="se", bufs=1,
                                         name="se_ps")
                        for jc in range(NJC):
                            nc.tensor.matmul(
                                av_ps[:], v_sb[:, jc, h * P:(h + 1) * P],
                                attnT[:, jc, :],
                                start=(jc == 0), stop=(jc == NJC - 1))
                        for jc in range(NJC):
                            nc.tensor.matmul(
                                se_ps[:], ones_sb[:], attnT[:, jc, :],
                                start=(jc == 0), stop=(jc == NJC - 1))
                        recip_sb = atp.tile([1, IT], f32r, tag="recip",
                                            bufs=2, name="recip_sb")
                        with nc.allow_low_precision(
                                reason="fp22 recip of softmax denom is fine"):
                            nc.vector.reciprocal(recip_sb[:], se_ps[:])
                        bc_ps = psA.tile([P, IT], f32, tag="bc", bufs=1,
                                         name="bc_ps")
                        nc.tensor.matmul(bc_ps[:], onesk1_sb[:],
                                         recip_sb[:], start=True,
                                         stop=True)
                        bc_sb = atp.tile([P, IT], f32, tag="bc_sb", bufs=2,
                                         name="bc_sb")
                        nc.scalar.copy(bc_sb[:], bc_ps[:])
                        nc.vector.tensor_tensor(outT_sb[:, h, isl], av_ps[:],
                                                bc_sb[:], MULT)

                    # out-proj for the i-chunks of this i-tile
                    for ic2 in range(IT // P):
                        ic = it * (IT // P) + ic2
                        f_ps = psA.tile([P, 2, 512], f32, tag="sim", bufs=2,
                                        name="f_ps")
                        for h in range(NHEAD_CORE):
                            for nt in range(2):
                                nc.tensor.matmul(
                                    f_ps[:, nt, :],
                                    outT_sb[:, h, ic * P:(ic + 1) * P],
                                    wo_sb[:, h, nt * 512:(nt + 1) * 512],
                                    start=(h == 0),
                                    stop=(h == NHEAD_CORE - 1))
                        f_sb = atp.tile([P, NQ], f32, tag="f_sb", bufs=3,
                                        name="f_sb")
                        if ic % 2 == 0:
                            nc.vector.tensor_copy(out=f_sb[:], in_=f_ps[:])
                        else:
                            nc.scalar.copy(f_sb[:], f_ps[:])
                        nc.sync.dma_start(out[ic * P:(ic + 1) * P, :],
                                          f_sb[:])

    nc.compile()
    return nc


def get_program():
    if "nc" not in _CACHE:
        _CACHE["nc"] = _build_program()
    return _CACHE["nc"]


def _prep_in_maps(inputs):
    """Host-side sharding: core c -> (batch c//4, heads [2m, 2m+1], m=c%4)."""
    f32 = np.float32
    x = np.asarray(inputs["x"], f32)
    c1 = np.asarray(inputs["context"], f32)
    c2 = np.asarray(inputs["context2"], f32)
    m1 = np.asarray(inputs["mask1"])
    m2 = np.asarray(inputs["mask2"])
    scale = np.float32(DH ** -0.5)
    Wq = np.asarray(inputs["Wq"], f32) * scale
    bq = np.asarray(inputs["bq"], f32) * scale
    Wk1 = np.asarray(inputs["Wk1"], f32)
    bk1 = np.asarray(inputs["bk1"], f32)
    Wv1 = np.asarray(inputs["Wv1"], f32)
    bv1 = np.asarray(inputs["bv1"], f32)
    Wk2 = np.asarray(inputs["Wk2"], f32)
    bk2 = np.asarray(inputs["bk2"], f32)
    Wv2 = np.asarray(inputs["Wv2"], f32)
    bv2 = np.asarray(inputs["bv2"], f32)

    ac = np.ascontiguousarray
    xT = [ac(x[b].T) for b in range(2)]
    cT = [ac(np.concatenate([c1[b].T, c2[b].T], axis=1)) for b in range(2)]
    maskT = [ac(np.concatenate([m1[b].T, m2[b].T], axis=0)).astype(np.uint8)
             for b in range(2)]

    in_maps = []
    for c in range(N_CORES):
        b, m = c // 4, c % 4
        ksl = slice(m * P, (m + 1) * P)          # 128 k-cols (2 heads x 64)
        vsl = slice(m * 2 * P, (m + 1) * 2 * P)  # 256 v-cols (2 heads x 128)
        in_maps.append({
            "xT": xT[b],
            "cT": cT[b],
            "maskT": maskT[b],
            "wq": ac(Wq[:, ksl]),
            "wk1": ac(Wk1[:, ksl]),
            "wk2": ac(Wk2[:, ksl]),
            "wv1": ac(Wv1[:, vsl]),
            "wv2": ac(Wv2[:, vsl]),
            "wo": ac(inputs["Wo"][vsl, :]).astype(f32),
            "bq": ac(bq[ksl, None]),
            "bk1": ac(bk1[ksl, None]),
            "bk2": ac(bk2[ksl, None]),
            "bv": ac(np.concatenate([bv1[vsl], bv2[vsl]])[None, :]),
            "ones_c": np.ones((P, 1), f32),
            "ones_r": np.ones((1, P), f32),
        })
    return in_maps


def run_sharded(inputs, trace=False, **kw):
    """Compile+run on 8 cores; returns (full_output, BassKernelResults)."""
    from concourse import bass_utils
    nc = get_program()
    in_maps = _prep_in_maps(inputs)
    res = bass_utils.run_bass_kernel_spmd(
        nc, in_maps, core_ids=list(range(N_CORES)), trace=trace, **kw)
    bo = np.asarray(inputs["bo"], np.float32)
    out = np.zeros((2, NQ, NQ), np.float32)
    for c in range(N_CORES):
        out[c // 4] += res.results[c]["out"]
    out += bo[None, None, :]
    return out, res


def kernel(**inputs):
    out, _ = run_sharded(inputs, trace=False)
    return out


# revision 5
# speedup vs baseline: 1.2082x; 1.2082x over previous
"""Trainium2 Bass kernel for nn_Attention2Context (2-context masked attention).

Self-contained: builds one SPMD Bass/Tile program, shards the FULL inputs
across 8 NeuronCores host-side (batch x 2-head groups; tensor-parallel heads:
q/k/v column-parallel, out-proj row-parallel), runs via
concourse.bass_utils.run_bass_kernel_spmd, and reduces the row-parallel
partial outputs host-side.

Math notes:
- softmax max-subtraction is skipped: logits are |sim| << 1 by construction
  (inputs scaled 0.02), masked positions are multiplied by 0 AFTER exp, which
  is exactly softmax(where(mask, sim, -inf)) when not all-masked.
- the 1/sqrt(DH) scale is folded into Wq/bq host-side.
- v biases are added on-chip (broadcast via K=1 matmul); bo is added on the
  host after the partial-sum reduction.
- all matmuls run as float32r (fp22 multiplies, fp32 accumulate).
"""

import sys

for _p in ("/opt/trn_rl_repo", "/root/.axon_site/_ro/trn_rl_repo"):
    if _p not in sys.path:
        sys.path.append(_p)

import numpy as np

P = 128
NQ = 1024
J = 2048          # NC1 + NC2
QD = 1024
NKC = 8           # contraction chunks (QD / P)
NJC = 16          # context chunks (J / P)
IT = 512          # i-tile width
NIT = NQ // IT    # 2
NHEAD_CORE = 2    # heads per core
DH = 64
N_CORES = 8

_CACHE = {}


def _build_program():
    import concourse.mybir as mybir
    import concourse.tile as tile
    from concourse import bacc

    f32 = mybir.dt.float32
    f32r = mybir.dt.float32r
    u8 = mybir.dt.uint8
    EXP = mybir.ActivationFunctionType.Exp
    MULT = mybir.AluOpType.mult
    ADD = mybir.AluOpType.add

    nc = bacc.Bacc("TRN2", target_bir_lowering=False, debug=False,
                   num_devices=N_CORES)

    def din(name, shape, dt=f32):
        return nc.dram_tensor(name, shape, dt, kind="ExternalInput").ap()

    xT = din("xT", [QD, NQ], f32r)           # x[b].T
    cT = din("cT", [QD, J], f32r)            # [ctx1[b].T | ctx2[b].T]
    maskT = din("maskT", [J, NQ], u8)        # [m1[b].T ; m2[b].T]
    wq = din("wq", [QD, P], f32r)            # pre-scaled by DH**-0.5
    wk1 = din("wk1", [QD, P], f32r)
    wk2 = din("wk2", [QD, P], f32r)
    wv1 = din("wv1", [QD, 2 * P], f32r)
    wv2 = din("wv2", [QD, 2 * P], f32r)
    wo = din("wo", [2 * P, NQ], f32r)
    bq = din("bq", [P, 1])                   # pre-scaled by DH**-0.5
    bk1 = din("bk1", [P, 1])
    bk2 = din("bk2", [P, 1])
    bv = din("bv", [1, 2 * 2 * P], f32r)     # [bv1_slice | bv2_slice]
    ones_c = din("ones_c", [P, 1], f32r)     # all-ones column
    ones_r = din("ones_r", [1, P], f32r)     # all-ones row
    out = nc.dram_tensor("out", [NQ, NQ], f32, kind="ExternalOutput").ap()


    with tile.TileContext(nc) as tc:
        with tc.tile_pool(name="persist", bufs=1) as pp:
            mask_sb = pp.tile([P, NJC, NQ], u8, tag="mask", name="mask_sb")
            qT_sb = pp.tile([P, NQ], f32r, tag="qT", name="qT_sb")
            kT_sb = pp.tile([P, J], f32r, tag="kT", name="kT_sb")
            v_sb = pp.tile([P, NJC, 2 * P], f32r, tag="v", name="v_sb")
            wo_sb = pp.tile([P, 2, NQ], f32r, tag="wo", name="wo_sb")
            outT_sb = pp.tile([P, 2, NQ], f32r, tag="outT", name="outT_sb")
            bq_sb = pp.tile([P, 1], f32, tag="bq", name="bq_sb")
            bk1_sb = pp.tile([P, 1], f32, tag="bk1", name="bk1_sb")
            bk2_sb = pp.tile([P, 1], f32, tag="bk2", name="bk2_sb")
            ones_sb = pp.tile([P, 1], f32r, tag="ones", name="ones_sb")
            onesk1_sb = pp.tile([1, P], f32r, tag="onesk1", name="onesk1_sb")
            bv_sb = pp.tile([1, 4 * P], f32r, tag="bv", name="bv_sb")
            bvb_sb = pp.tile([P, 4 * P], f32, tag="bvb", name="bvb_sb")

            nc.sync.dma_start(bq_sb[:], bq)
            nc.sync.dma_start(bk1_sb[:], bk1)
            nc.sync.dma_start(bk2_sb[:], bk2)
            nc.sync.dma_start(bv_sb[:], bv)
            nc.sync.dma_start(ones_sb[:], ones_c)
            nc.sync.dma_start(onesk1_sb[:], ones_r)

            # broadcast v biases to all 128 partitions: ones_col.T @ bv_row
            with tc.tile_pool(name="psB", bufs=1, space="PSUM") as psB:
                bvb_ps = psB.tile([P, 4 * P], f32, tag="bvb_ps",
                                  name="bvb_ps")
                nc.tensor.matmul(bvb_ps[:], onesk1_sb[:], bv_sb[:],
                                 start=True, stop=True)
                nc.scalar.copy(bvb_sb[:], bvb_ps[:])

            # ---------------- phase 1: projections ----------------
            with tc.tile_pool(name="proj", bufs=1) as prj, \
                 tc.tile_pool(name="projs", bufs=3) as prjs, \
                 tc.tile_pool(name="psP", bufs=1, space="PSUM") as psP:
                cT_sb = prj.tile([P, NKC, J], f32r, tag="cT", name="cT_sb")
                wq_sb = prj.tile([P, NKC, P], f32r, tag="wq", name="wq_sb")
                wk1_sb = prj.tile([P, NKC, P], f32r, tag="wk1", name="wk1_sb")
                wk2_sb = prj.tile([P, NKC, P], f32r, tag="wk2", name="wk2_sb")
                wv1_sb = prj.tile([P, NKC, 2 * P], f32r, tag="wv1",
                                  name="wv1_sb")
                wv2_sb = prj.tile([P, NKC, 2 * P], f32r, tag="wv2",
                                  name="wv2_sb")

                nc.sync.dma_start(wq_sb[:],
                                  wq.rearrange("(kc p) m -> p kc m", p=P))
                nc.sync.dma_start(wk1_sb[:],
                                  wk1.rearrange("(kc p) m -> p kc m", p=P))
                nc.sync.dma_start(wk2_sb[:],
                                  wk2.rearrange("(kc p) m -> p kc m", p=P))

                # qT / kT: kc-outer so matmuls chase the cT/xT DMAs
                q_ps = [psP.tile([P, 512], f32, tag="qk_ps", bufs=6,
                                 name=f"q_ps{nt}") for nt in range(2)]
                k_ps = [psP.tile([P, 512], f32, tag="qk_ps", bufs=6,
                                 name=f"k_ps{nt}") for nt in range(4)]
                xts = []
                for kc in range(NKC):
                    xt = prjs.tile([P, NQ], f32r, tag="xt", bufs=3, name="xt")
                    xts.append(xt)
                    nc.sync.dma_start(xt[:], xT[kc * P:(kc + 1) * P, :])
                    nc.sync.dma_start(cT_sb[:, kc, :],
                                      cT[kc * P:(kc + 1) * P, :])
                nc.sync.dma_start(wv1_sb[:],
                                  wv1.rearrange("(kc p) m -> p kc m", p=P))
                nc.sync.dma_start(wv2_sb[:],
                                  wv2.rearrange("(kc p) m -> p kc m", p=P))
                for jc in range(NJC):
                    nc.sync.dma_start(mask_sb[:, jc, :],
                                      maskT[jc * P:(jc + 1) * P, :])
                nc.sync.dma_start(wo_sb[:],
                                  wo.rearrange("(h p) o -> p h o", p=P))
                for kc in range(NKC):
                    xt = xts[kc]
                    for nt in range(2):
                        nc.tensor.matmul(
                            q_ps[nt][:], wq_sb[:, kc, :],
                            xt[:, nt * 512:(nt + 1) * 512],
                            start=(kc == 0), stop=(kc == NKC - 1))
                    for nt in range(4):
                        wk_sb = wk1_sb if nt < 2 else wk2_sb
                        nc.tensor.matmul(
                            k_ps[nt][:], wk_sb[:, kc, :],
                            cT_sb[:, kc, nt * 512:(nt + 1) * 512],
                            start=(kc == 0), stop=(kc == NKC - 1))
                for nt in range(2):
                    nc.scalar.add(qT_sb[:, nt * 512:(nt + 1) * 512],
                                  q_ps[nt][:], bq_sb[:])
                for nt in range(4):
                    bk_sb = bk1_sb if nt < 2 else bk2_sb
                    nc.scalar.add(kT_sb[:, nt * 512:(nt + 1) * 512],
                                  k_ps[nt][:], bk_sb[:])

                # v: [J, 256], j on partitions, + bias via broadcast tile
                for jc in range(NJC):
                    wv_sb = wv1_sb if jc < NJC // 2 else wv2_sb
                    bvb_half = (bvb_sb[:, 0:256] if jc < NJC // 2
                                else bvb_sb[:, 256:512])
                    v_ps = psP.tile([P, 2 * P], f32, tag="v_ps", bufs=2,
                                    name="v_ps")
                    for kc in range(NKC):
                        nc.tensor.matmul(
                            v_ps[:], cT_sb[:, kc, jc * P:(jc + 1) * P],
                            wv_sb[:, kc, :],
                            start=(kc == 0), stop=(kc == NKC - 1))
                    nc.vector.tensor_tensor(v_sb[:, jc, :], v_ps[:],
                                            bvb_half, ADD)

            # ---------------- phase 2+3: attention + out-proj ----------
            with tc.tile_pool(name="attn", bufs=1) as atp, \
                 tc.tile_pool(name="psA", bufs=1, space="PSUM") as psA:
                for it in range(NIT):
                    isl = slice(it * IT, (it + 1) * IT)
                    for h in range(NHEAD_CORE):
                        hsl = slice(h * DH, (h + 1) * DH)
                        blk = it * NHEAD_CORE + h
                        mask_eng = nc.vector if blk % 2 == 0 else nc.gpsimd
                        attnT = atp.tile([P, NJC, IT], f32r, tag="attnT",
                                         bufs=2, name="attnT")
                        for jg in range(NJC // 2):
                            sim_ps = psA.tile([P, 2, IT], f32, tag="sim",
                                              bufs=2, name="sim_ps")
                            for g in range(2):
                                jc = jg * 2 + g
                                nc.tensor.matmul(
                                    sim_ps[:, g, :],
                                    kT_sb[hsl, jc * P:(jc + 1) * P],
                                    qT_sb[hsl, isl],
                                    start=True, stop=True)
                            nc.scalar.activation(
                                attnT[:, jg * 2:jg * 2 + 2, :], sim_ps[:],
                                EXP)
                            mask_eng.tensor_tensor(
                                attnT[:, jg * 2:jg * 2 + 2, :],
                                attnT[:, jg * 2:jg * 2 + 2, :],
                                mask_sb[:, jg * 2:jg * 2 + 2, isl], MULT)

                        av_ps = psA.tile([P, IT], f32, tag="av", bufs=2,
                                         name="av_ps")
                        se_ps = psA.tile([1, IT], f32, tag="se", bufs=1,
                                         name="se_ps")
                        for jc in range(NJC):
                            nc.tensor.matmul(
                                av_ps[:], v_sb[:, jc, h * P:(h + 1) * P],
                                attnT[:, jc, :],
                                start=(jc == 0), stop=(jc == NJC - 1))
                        for jc in range(NJC):
                            nc.tensor.matmul(
                                se_ps[:], ones_sb[:], attnT[:, jc, :],
                                start=(jc == 0), stop=(jc == NJC - 1))
                        recip_f32 = atp.tile([1, IT], f32, tag="recipf",
                                             bufs=2, name="recip_f32")
                        recip_sb = atp.tile([1, IT], f32r, tag="recip",
                                            bufs=2, name="recip_sb")
                        nc.vector.reciprocal_approx_fast(recip_f32[:],
                                                         se_ps[:])
                        with nc.allow_low_precision(
                                reason="fp22 recip of softmax denom is fine"):
                            nc.vector.tensor_copy(out=recip_sb[:],
                                                  in_=recip_f32[:])
                        bc_ps = psA.tile([P, IT], f32, tag="bc", bufs=1,
                                         name="bc_ps")
                        nc.tensor.matmul(bc_ps[:], onesk1_sb[:],
                                         recip_sb[:], start=True,
                                         stop=True)
                        bc_sb = atp.tile([P, IT], f32, tag="bc_sb", bufs=2,
                                         name="bc_sb")
                        nc.scalar.copy(bc_sb[:], bc_ps[:])
                        nc.vector.tensor_tensor(outT_sb[:, h, isl], av_ps[:],
                                                bc_sb[:], MULT)

                    # out-proj for the i-chunks of this i-tile
                    for ic2 in range(IT // P):
                        ic = it * (IT // P) + ic2
                        f_ps = psA.tile([P, 2, 512], f32, tag="sim", bufs=2,
                                        name="f_ps")
                        for h in range(NHEAD_CORE):
                            for nt in range(2):
                                nc.tensor.matmul(
                                    f_ps[:, nt, :],
                                    outT_sb[:, h, ic * P:(ic + 1) * P],
                                    wo_sb[:, h, nt * 512:(nt + 1) * 512],
                                    start=(h == 0),
                                    stop=(h == NHEAD_CORE - 1))
                        f_sb = atp.tile([P, NQ], f32, tag="f_sb", bufs=3,
                                        name="f_sb")
                        if ic % 2 == 0:
                            nc.vector.tensor_copy(out=f_sb[:], in_=f_ps[:])
                        else:
                            nc.scalar.copy(f_sb[:], f_ps[:])
                        nc.sync.dma_start(out[ic * P:(ic + 1) * P, :],
                                          f_sb[:])

    nc.compile()
    return nc


def get_program():
    if "nc" not in _CACHE:
        _CACHE["nc"] = _build_program()
    return _CACHE["nc"]


def _prep_in_maps(inputs):
    """Host-side sharding: core c -> (batch c//4, heads [2m, 2m+1], m=c%4)."""
    f32 = np.float32
    x = np.asarray(inputs["x"], f32)
    c1 = np.asarray(inputs["context"], f32)
    c2 = np.asarray(inputs["context2"], f32)
    m1 = np.asarray(inputs["mask1"])
    m2 = np.asarray(inputs["mask2"])
    scale = np.float32(DH ** -0.5)
    Wq = np.asarray(inputs["Wq"], f32) * scale
    bq = np.asarray(inputs["bq"], f32) * scale
    Wk1 = np.asarray(inputs["Wk1"], f32)
    bk1 = np.asarray(inputs["bk1"], f32)
    Wv1 = np.asarray(inputs["Wv1"], f32)
    bv1 = np.asarray(inputs["bv1"], f32)
    Wk2 = np.asarray(inputs["Wk2"], f32)
    bk2 = np.asarray(inputs["bk2"], f32)
    Wv2 = np.asarray(inputs["Wv2"], f32)
    bv2 = np.asarray(inputs["bv2"], f32)

    ac = np.ascontiguousarray
    xT = [ac(x[b].T) for b in range(2)]
    cT = [ac(np.concatenate([c1[b].T, c2[b].T], axis=1)) for b in range(2)]
    maskT = [ac(np.concatenate([m1[b].T, m2[b].T], axis=0)).astype(np.uint8)
             for b in range(2)]

    in_maps = []
    for c in range(N_CORES):
        b, m = c // 4, c % 4
        ksl = slice(m * P, (m + 1) * P)          # 128 k-cols (2 heads x 64)
        vsl = slice(m * 2 * P, (m + 1) * 2 * P)  # 256 v-cols (2 heads x 128)
        in_maps.append({
            "xT": xT[b],
            "cT": cT[b],
            "maskT": maskT[b],
            "wq": ac(Wq[:, ksl]),
            "wk1": ac(Wk1[:, ksl]),
            "wk2": ac(Wk2[:, ksl]),
            "wv1": ac(Wv1[:, vsl]),
            "wv2": ac(Wv2[:, vsl]),
            "wo": ac(inputs["Wo"][vsl, :]).astype(f32),
            "bq": ac(bq[ksl, None]),
            "bk1": ac(bk1[ksl, None]),
            "bk2": ac(bk2[ksl, None]),
            "bv": ac(np.concatenate([bv1[vsl], bv2[vsl]])[None, :]),
            "ones_c": np.ones((P, 1), f32),
            "ones_r": np.ones((1, P), f32),
        })
    return in_maps


def run_sharded(inputs, trace=False, **kw):
    """Compile+run on 8 cores; returns (full_output, BassKernelResults)."""
    from concourse import bass_utils
    nc = get_program()
    in_maps = _prep_in_maps(inputs)
    res = bass_utils.run_bass_kernel_spmd(
        nc, in_maps, core_ids=list(range(N_CORES)), trace=trace, **kw)
    bo = np.asarray(inputs["bo"], np.float32)
    out = np.zeros((2, NQ, NQ), np.float32)
    for c in range(N_CORES):
        out[c // 4] += res.results[c]["out"]
    out += bo[None, None, :]
    return out, res


def kernel(**inputs):
    out, _ = run_sharded(inputs, trace=False)
    return out
